# revision 13
# baseline (speedup 1.0000x reference)
"""DenseCRF mean-field inference on 8 Trainium2 NeuronCores.

Math: per image (1 here), 5 iterations of
    q_hat = U + 4*((q/n) @ K)/n + 2*(q @ S);  q = softmax(q_hat, axis=0)
with K[i,j] = exp(-0.5*d2(i,j)) the dense 9216x9216 bilateral kernel,
n = sqrt(colsum K) and S the (input-independent) spatial kernel matrix.
S's band cutoff at |d|=35 only zeroes values ~1e-10, so S is treated as an
exact 2D Gaussian and generated ON DEVICE via a second feature matmul + exp
(the baseline shipped it as a 21 MB/core host input = 170 MB of H2D per
call, which dominated measured time).

Fold:  Mt[i,j] = beta*(4K[i,j]*rn_i + 2S[i,j]/rn_j),  rn = 1/sqrt(colsum K)
       q_hat = (q*gamma) @ Mt * (rn_j/(beta*gamma)) + U
Mt's columns (output pixels j) are sharded over the 8 cores; each core keeps
Mt[:, mine] = [9216, 1152] resident in SBUF in fp8-e4m3 (10.6 MB), with q in
fp8 too so the per-iteration matvec runs in DoubleRow mode (2x PE rate).
beta=32 / gamma=128 are exact power-of-2 boosts that keep small kernel
entries out of the fp8 flush zone; both are divided back out in the f32
post-scale.  Full-fidelity numpy sim of this pipeline: rel L2 3.0e-3 vs
the 2e-2 gate.

Build:
  pass 1:  E = exp(T1) via a 13-row f32r feature matmul (features centered
           and pre-rounded to 13 mantissa bits on host so the FP22 truncation
           is backward-stable) + ACT exp straight into the fp8 tile.
  colsum:  ones-matmul over the fp8 tile in DoubleRow (36 pair-matmuls),
           rn rows from ACT ln/exp with fused scale+bias; AllGather rn
           (37 KB) for the row side; -ln(rn_j) written into the rhs row
           bank for pass 2.
  pass 2:  T2' = spatial feature matmul (5 f32r rows incl. the -ln(rn_j)
           row) + exp -> e2 = beta*2S/rn_j in bf16; one fused
           scalar_tensor_tensor per chunk: Mt = (Mt*rn_i) + e2 (split
           DVE/GPSIMD).  Overlaps the rn AllGather (only the STT needs it).
Iteration: 36 DoubleRow pair-matmuls accumulate [21,1152] in PSUM; +U and
rn_j post-scale on DVE; softmax over 21 classes via PE transpose + ACT exp
with accum_out + reciprocal; AllGather of the fp8 q shard (24 KB).  The
iteration-0 softmax (softmax(U)) and its AllGather are issued before pass 1
so that collective hides under the build.
"""

import numpy as np

H = 96
W = 96
P = H * W            # 9216 pixels
L = 21               # classes
NCORES = 8
PSH = P // NCORES    # 1152 pixels per core
NI = P // 128        # 72 contraction chunks
NP = NI // 2         # 36 DoubleRow chunk pairs
NJ = PSH // 128      # 9 output-pixel chunks per core
NSLAB = 8            # lhsT feature slabs of 1152 columns
SXY_BF = 70.0
SC_BF = 12.0
SXY_SP = 6.0
BETA = 32.0          # fp8 magnitude boost for Mt
GAMMA = 128.0        # fp8 magnitude boost for q
NROW1 = 13           # T1 (bilateral) contraction rows
NROW2 = 5            # T2 (spatial) contraction rows
SLICES = ((0, 512), (512, 512), (1024, 128))

_CACHE = {}
TRACE = False      # set by test harness for profiling runs
LAST_RESULT = None


# ----------------------------------------------------------------------------
# host-side operand rows
# ----------------------------------------------------------------------------

def _m13(x):
    # round to 13 explicit mantissa bits: exactly representable in FP22, so
    # the PE's f32r truncation is a no-op and products are exact in f32 accum
    m, e = np.frexp(np.asarray(x, np.float64))
    return np.ldexp(np.round(m * (1 << 14)) / (1 << 14), e)


def _host_rows(ref):
    """PL [18, P] (lhsT source, full pixels) and QR [18, P] (rhs source).

    T1 rows 0-12:  f_i.f_j - 0.5|f_i|^2 - 0.5|f_j|^2 + ln4 + ln(beta)
    T2 rows 13-17: -(dy^2+dx^2)/72 + ln(2*beta/z^2) - ln(rn_j)
    (row 17's rhs side, -ln(rn_j), is filled in on device after colsum)
    """
    ys = np.arange(P) // W
    xs = np.arange(P) % W
    # centering leaves all pairwise differences (and hence K, S) unchanged
    # but shrinks feature magnitudes so FP22-rounded products stay accurate
    yc, xc = ys - 47.5, xs - 47.5
    c = np.asarray(ref, np.float64).reshape(3, P) - 127.5

    g1 = np.exp(-((np.arange(71.0) - 35.0) ** 2) / 72.0)
    z = g1.sum()

    yf, xf = _m13(yc / SXY_BF), _m13(xc / SXY_BF)
    cf = _m13(c / SC_BF)
    syxC = _m13(-0.5 * (yf ** 2 + xf ** 2) + np.log(4.0) + np.log(BETA))
    syxQ = _m13(-0.5 * (yf ** 2 + xf ** 2))
    cd = _m13(-0.5 * cf ** 2)
    y6, x6 = _m13(yc / SXY_SP), _m13(xc / SXY_SP)
    sdC = _m13(-0.5 * (y6 ** 2 + x6 ** 2) + np.log(2.0 * BETA)
               - 2.0 * np.log(z))
    sdQ = _m13(-0.5 * (y6 ** 2 + x6 ** 2))
    one = np.ones(P)
    zero = np.zeros(P)

    pl = np.stack([yf, xf, *cf, syxC, *cd, one, one, one, one,
                   y6, x6, sdC, one, one], 0)
    qr = np.stack([yf, xf, *cf, one, one, one, one, syxQ, *cd,
                   y6, x6, one, sdQ, zero], 0)
    return (np.ascontiguousarray(pl, dtype=np.float32),
            np.ascontiguousarray(qr, dtype=np.float32))


# ----------------------------------------------------------------------------
# device program
# ----------------------------------------------------------------------------

def _build_bass(niters=5):
    key = ("nc", niters)
    if key in _CACHE:
        return _CACHE[key]

    import concourse.bass as bass
    import concourse.bacc as bacc
    import concourse.tile as tile
    import concourse.mybir as mybir
    from concourse.masks import make_identity

    f32 = mybir.dt.float32
    f32r = mybir.dt.float32r
    bf16 = mybir.dt.bfloat16
    fp8 = mybir.dt.float8e4
    AF = mybir.ActivationFunctionType
    ALU = mybir.AluOpType
    DR = mybir.MatmulPerfMode.DoubleRow

    LN4B = float(np.log(4.0 * BETA))
    LNBG = float(np.log(BETA * GAMMA))

    nc = bacc.Bacc("TRN2", num_devices=NCORES)

    pl = nc.dram_tensor("pl", [18, P], f32r, kind="ExternalInput")
    qr = nc.dram_tensor("qr", [18, PSH], f32r, kind="ExternalInput")
    unary_m = nc.dram_tensor("unary_m", [L, PSH], f32, kind="ExternalInput")
    qout = nc.dram_tensor("qout", [NJ, 128, L], f32, kind="ExternalOutput")

    rg = [list(range(NCORES))]

    with tile.TileContext(nc) as tc:
        with tc.tile_pool(name="dram", bufs=1, space="DRAM") as dram:
            rn_in_d = dram.tile([1, PSH], f32)      # my rn, AG input
            rn_out_d = dram.tile([NI, 128], f32)    # full rn, AG output
            rnp_d = dram.tile([1, PSH], f32)        # my rn_j/(beta*gamma)
            qsh_d = dram.tile([NJ, 128, L], fp8)    # q shard, AG input
            qfl_d = dram.tile([NI, 128, L], fp8)    # full q, AG output

            with tc.tile_pool(name="persist", bufs=1) as persist:
                asmQR = persist.tile([NROW1, PSH], f32r)
                nc.sync.dma_start(asmQR[:], qr[0:NROW1, :])
                asmQR2 = persist.tile([NROW2, PSH], f32r)
                nc.sync.dma_start(asmQR2[0:NROW2 - 1, :],
                                  qr[NROW1:NROW1 + NROW2 - 1, :])
                ident = persist.tile([L, L], f32)
                make_identity(nc, ident[:])
                U_sb = persist.tile([L, PSH], f32)
                rnI = persist.tile([128, NI], f32)
                rnJ21 = persist.tile([L, PSH], f32)
                # DoubleRow weights need pair-dim stride % 16 == 0
                ones2 = persist.tile([128, 2, 16], fp8)
                nc.vector.memset(ones2[:], 1.0)

                mpool = tc.tile_pool(name="mres", bufs=1)
                mpool_h = mpool.__enter__()
                Mt = mpool_h.tile([128, NP, 2, PSH], fp8, name="Mt")

                with (
                    tc.tile_pool(name="itq", bufs=1) as itq,
                    tc.tile_pool(name="ite", bufs=4) as ite,
                    tc.tile_pool(name="ittp", bufs=2, space="PSUM") as ittp,
                ):
                    # U = ln(clip(u))
                    ut = itq.tile([L, PSH], f32, tag="ut")
                    nc.sync.dma_start(ut[:], unary_m[:, :])
                    nc.vector.tensor_scalar(ut[:], ut[:], 1e-5, 1.0,
                                            op0=ALU.max, op1=ALU.min)
                    nc.scalar.activation(U_sb[:], ut[:], AF.Ln)

                    def softmax_stage(qh_cur, it):
                        last = it == niters
                        if not last:
                            qm = itq.tile([128, NJ, L], fp8, tag="qm")
                        else:
                            qo = itq.tile([128, NJ, L], f32, tag="qo")
                        zz = ite.tile([128, NJ], f32, tag="zz")
                        rz = ite.tile([128, NJ], f32, tag="rz")
                        for jc in range(NJ):
                            tp = ittp.tile([128, L], f32, tag="tp")
                            nc.tensor.transpose(
                                tp[:], qh_cur[:, jc * 128:(jc + 1) * 128],
                                ident[:])
                            e = ite.tile([128, L], f32, tag="e")
                            nc.scalar.activation(
                                e[:], tp[:], AF.Exp,
                                accum_out=zz[:, jc:jc + 1])
                            nc.vector.reciprocal(rz[:, jc:jc + 1],
                                                 zz[:, jc:jc + 1])
                            if not last:
                                nc.vector.tensor_scalar(
                                    qm[:, jc, :], e[:], rz[:, jc:jc + 1],
                                    GAMMA, op0=ALU.mult, op1=ALU.mult)
                            else:
                                nc.vector.tensor_scalar_mul(
                                    qo[:, jc, :], e[:], rz[:, jc:jc + 1])
                        if not last:
                            nc.gpsimd.dma_start(
                                qsh_d.rearrange("a b c -> b a c"), qm[:])
                            nc.gpsimd.collective_compute(
                                "AllGather", ALU.bypass, replica_groups=rg,
                                ins=[qsh_d.rearrange("a b c -> (a b c)")],
                                outs=[qfl_d.rearrange("a b c -> (a b c)")])
                            # L padded to 32 so the DoubleRow pair-dim
                            # stride (32) is a multiple of 16
                            qf = itq.tile([128, NI, 32], fp8, tag="qf",
                                          bufs=2)
                            nc.sync.dma_start(
                                qf[:, :, 0:L],
                                qfl_d.rearrange("a b c -> b a c"))
                            return qf
                        nc.gpsimd.dma_start(
                            qout[:, :, :].rearrange("a b c -> b a c"), qo[:])
                        return None

                    # iteration-0 softmax + AllGather overlap the build
                    qf_cur = softmax_stage(U_sb, 0)

                    # ---- pass 1: E = exp(T1) into the fp8 resident tile --
                    with (
                        tc.tile_pool(name="slab", bufs=2) as slabp,
                        tc.tile_pool(name="eps", bufs=2, space="PSUM") as eps,
                    ):
                        for sb in range(NSLAB):
                            sl = slabp.tile([NROW1, PSH], f32r, tag="sl")
                            nc.sync.dma_start(
                                sl[:], pl[0:NROW1, sb * PSH:(sb + 1) * PSH])
                            for k in range(NI // NSLAB):
                                ic = sb * (NI // NSLAB) + k
                                ps = eps.tile([128, PSH], f32, tag="eps")
                                lh = sl[:, k * 128:(k + 1) * 128]
                                for (o, n) in SLICES:
                                    nc.tensor.matmul(
                                        ps[:, o:o + n],
                                        lh,
                                        asmQR[0:NROW1, o:o + n],
                                        start=True, stop=True)
                                nc.scalar.activation(
                                    Mt[:, ic // 2, ic % 2, :], ps[:], AF.Exp)

                    # ---- colsum (DoubleRow over fp8) -> rn rows ----------
                    with (
                        tc.tile_pool(name="csp", bufs=1, space="PSUM") as csp,
                        tc.tile_pool(name="cst1", bufs=1) as cst1,
                    ):
                        cs = csp.tile([1, PSH], f32)
                        for k in range(NP):
                            for (o, n) in SLICES:
                                nc.tensor.matmul(cs[:, o:o + n],
                                                 ones2[:, :, 0:1],
                                                 Mt[:, k, :, o:o + n],
                                                 start=(k == 0),
                                                 stop=(k == NP - 1),
                                                 perf_mode=DR)
                        lncs = cst1.tile([1, PSH], f32)
                        nc.scalar.activation(lncs[:], cs[:], AF.Ln)
                        # -ln(rn_j) = 0.5*ln(cs) - 0.5*ln(4*beta)
                        qrow17 = cst1.tile([1, PSH], f32)
                        nc.vector.tensor_scalar(qrow17[:], lncs[:],
                                                0.5, -0.5 * LN4B,
                                                op0=ALU.mult, op1=ALU.add)
                        nc.sync.dma_start(asmQR2[NROW2 - 1:NROW2, :],
                                          qrow17[:].bitcast(f32r))
                        rnrow = cst1.tile([1, PSH], f32)
                        nc.vector.tensor_scalar(rnrow[:], lncs[:],
                                                -0.5, 0.5 * LN4B,
                                                op0=ALU.mult, op1=ALU.add)
                        nc.scalar.activation(rnrow[:], rnrow[:], AF.Exp)
                        nc.gpsimd.dma_start(rn_in_d[:, :], rnrow[:])
                        rnpost = cst1.tile([1, PSH], f32)
                        nc.vector.tensor_scalar(rnpost[:], lncs[:],
                                                -0.5, 0.5 * LN4B - LNBG,
                                                op0=ALU.mult, op1=ALU.add)
                        nc.scalar.activation(rnpost[:], rnpost[:], AF.Exp)
                        nc.gpsimd.dma_start(rnp_d[:, :], rnpost[:])

                    nc.gpsimd.collective_compute(
                        "AllGather", ALU.bypass, replica_groups=rg,
                        ins=[rn_in_d.rearrange("a b -> (a b)")],
                        outs=[rn_out_d.rearrange("a b -> (a b)")])
                    nc.sync.dma_start(rnI[:], rn_out_d.rearrange("a b -> b a"))
                    nc.sync.dma_start(rnJ21[:],
                                      rnp_d[0:1, :].to_broadcast((L, PSH)))

                    # ---- pass 2: Mt = (E * rn_i) + exp(T2') --------------
                    # T2' matmuls/exp depend only on the local colsum (via
                    # asmQR row 17); only the STT waits for the rn AllGather
                    with (
                        tc.tile_pool(name="slab2", bufs=2) as slab2,
                        tc.tile_pool(name="ps2p", bufs=2,
                                     space="PSUM") as ps2p,
                        tc.tile_pool(name="e2p", bufs=3) as e2p,
                    ):
                        for sb in range(NSLAB):
                            sl2 = slab2.tile([NROW2, PSH], f32r, tag="sl2")
                            nc.sync.dma_start(
                                sl2[:],
                                pl[NROW1:NROW1 + NROW2,
                                   sb * PSH:(sb + 1) * PSH])
                            for k in range(NI // NSLAB):
                                ic = sb * (NI // NSLAB) + k
                                ps2 = ps2p.tile([128, PSH], f32, tag="ps2")
                                lh2 = sl2[:, k * 128:(k + 1) * 128]
                                for (o, n) in SLICES:
                                    nc.tensor.matmul(
                                        ps2[:, o:o + n],
                                        lh2,
                                        asmQR2[:, o:o + n],
                                        start=True, stop=True)
                                e2 = e2p.tile([128, PSH], bf16, tag="e2")
                                nc.scalar.activation(e2[:], ps2[:], AF.Exp)
                                mt_c = Mt[:, ic // 2, ic % 2, :]
                                if ic % 3 < 2:
                                    nc.vector.scalar_tensor_tensor(
                                        mt_c, mt_c, rnI[:, ic:ic + 1], e2[:],
                                        op0=ALU.mult, op1=ALU.add)
                                else:
                                    nc.gpsimd.tensor_scalar_mul(
                                        mt_c, mt_c, rnI[:, ic:ic + 1])
                                    nc.gpsimd.tensor_add(mt_c, mt_c, e2[:])

                    # ---- iterations (DoubleRow matvec) -------------------
                    with tc.tile_pool(name="itps", bufs=1,
                                      space="PSUM") as itps:
                        for it in range(1, niters + 1):
                            ps = itps.tile([L, PSH], f32, tag="qbps")
                            for k in range(NP):
                                lhq = qf_cur[:, 2 * k:2 * k + 2, 0:L]
                                for (o, n) in SLICES:
                                    nc.tensor.matmul(
                                        ps[:, o:o + n], lhq,
                                        Mt[:, k, :, o:o + n],
                                        start=(k == 0),
                                        stop=(k == NP - 1),
                                        perf_mode=DR)
                            qh = itq.tile([L, PSH], f32, tag="qh", bufs=2)
                            nc.vector.tensor_tensor(qh[:], ps[:], rnJ21[:],
                                                    op=ALU.mult)
                            nc.vector.tensor_add(qh[:], qh[:], U_sb[:])
                            qf_cur = softmax_stage(qh, it)
                mpool.__exit__(None, None, None)

    nc.finalize()
    _CACHE[key] = nc
    return nc


# ----------------------------------------------------------------------------
# host entry point
# ----------------------------------------------------------------------------

def _in_maps(unary, ref):
    plf, qrf = _host_rows(ref)
    u2 = np.ascontiguousarray(np.asarray(unary, np.float32).reshape(L, P))
    maps = []
    for c in range(NCORES):
        sl = slice(c * PSH, (c + 1) * PSH)
        maps.append({
            "pl": plf,
            "qr": np.ascontiguousarray(qrf[:, sl]),
            "unary_m": np.ascontiguousarray(u2[:, sl]),
        })
    return maps


def kernel(unary: np.ndarray, ref: np.ndarray) -> np.ndarray:
    from concourse import bass_utils

    nc = _build_bass()
    in_maps = _in_maps(unary, ref)

    global LAST_RESULT
    res = bass_utils.run_bass_kernel_spmd(nc, in_maps,
                                          core_ids=list(range(NCORES)),
                                          trace=TRACE)
    LAST_RESULT = res
    shards = [res.results[c]["qout"].reshape(PSH, L) for c in range(NCORES)]
    qfull = np.concatenate(shards, 0)          # [P, L]
    out = qfull.T.reshape(1, L, H, W).astype(np.float32)
    return out


if __name__ == "__main__":
    u = np.random.rand(1, L, H, W).astype(np.float32)
    r = (np.random.rand(1, 3, H, W) * 255).astype(np.float32)
    o = kernel(u, r)
    print(o.shape, o.dtype, o.sum())


# revision 23
# speedup vs baseline: 1.3398x; 1.3398x over previous
"""DenseCRF mean-field inference on 8 Trainium2 NeuronCores.

Math: per image (1 here), 5 iterations of
    q_hat = U + 4*((q/n) @ K)/n + 2*(q @ S);  q = softmax(q_hat, axis=0)
with K[i,j] = exp(-0.5*d2(i,j)) the dense 9216x9216 bilateral kernel,
n = sqrt(colsum K) and S the (input-independent) spatial kernel matrix.
S's band cutoff at |d|=35 only zeroes values ~1e-10, so S is treated as an
exact 2D Gaussian and generated ON DEVICE via a second feature matmul + exp
(the baseline shipped it as a 21 MB/core host input = 170 MB of H2D per
call, which dominated measured time).

Fold:  Mt[i,j] = beta*(4K[i,j]*rn_i + 2S[i,j]/rn_j),  rn = 1/sqrt(colsum K)
       q_hat = (q*gamma) @ Mt * (rn_j/(beta*gamma)) + U
Mt's columns (output pixels j) are sharded over the 8 cores; each core keeps
Mt[:, mine] = [9216, 1152] resident in SBUF in fp8-e4m3 (10.6 MB), with q in
fp8 too so the per-iteration matvec runs in DoubleRow mode (2x PE rate).
beta=32 / gamma=128 are exact power-of-2 boosts that keep small kernel
entries out of the fp8 flush zone; both are divided back out in the f32
post-scale.  Full-fidelity numpy sim of this pipeline: rel L2 3.0e-3 vs
the 2e-2 gate.

Build:
  pass 1:  E = exp(T1) via a 13-row f32r feature matmul (features centered
           and pre-rounded to 13 mantissa bits on host so the FP22 truncation
           is backward-stable) + ACT exp straight into the fp8 tile.
  colsum:  ones-matmul over the fp8 tile in DoubleRow (36 pair-matmuls),
           rn rows from ACT ln/exp with fused scale+bias; AllGather rn
           (37 KB) for the row side; -ln(rn_j) written into the rhs row
           bank for pass 2.
  pass 2:  T2' = spatial feature matmul (5 f32r rows incl. the -ln(rn_j)
           row) + exp -> e2 = beta*2S/rn_j in bf16; one fused
           scalar_tensor_tensor per chunk: Mt = (Mt*rn_i) + e2 (split
           DVE/GPSIMD).  Overlaps the rn AllGather (only the STT needs it).
Iteration: 36 DoubleRow pair-matmuls accumulate [21,1152] in PSUM; +U and
rn_j post-scale on DVE; softmax over 21 classes via PE transpose + ACT exp
with accum_out + reciprocal; AllGather of the fp8 q shard (24 KB).  The
iteration-0 softmax (softmax(U)) and its AllGather are issued before pass 1
so that collective hides under the build.
"""

import numpy as np

H = 96
W = 96
P = H * W            # 9216 pixels
L = 21               # classes
NCORES = 8
PSH = P // NCORES    # 1152 pixels per core
NI = P // 128        # 72 contraction chunks
NP = NI // 2         # 36 DoubleRow chunk pairs
NJ = PSH // 128      # 9 output-pixel chunks per core
NSLAB = 8            # lhsT feature slabs of 1152 columns
SXY_BF = 70.0
SC_BF = 12.0
SXY_SP = 6.0
BETA = 32.0          # fp8 magnitude boost for Mt
GAMMA = 128.0        # fp8 magnitude boost for q
NROW1 = 13           # T1 (bilateral) contraction rows
NROW2 = 5            # T2 (spatial) contraction rows
SLICES = ((0, 512), (512, 512), (1024, 128))

_CACHE = {}
TRACE = False      # set by test harness for profiling runs
LAST_RESULT = None


# ----------------------------------------------------------------------------
# host-side operand rows
# ----------------------------------------------------------------------------

def _m13(x):
    # round to 13 explicit mantissa bits: exactly representable in FP22, so
    # the PE's f32r truncation is a no-op and products are exact in f32 accum
    m, e = np.frexp(np.asarray(x, np.float64))
    return np.ldexp(np.round(m * (1 << 14)) / (1 << 14), e)


def _host_rows(ref):
    """PL [18, P] (lhsT source, full pixels) and QR [18, P] (rhs source).

    T1 rows 0-12:  f_i.f_j - 0.5|f_i|^2 - 0.5|f_j|^2 + ln4 + ln(beta)
    T2 rows 13-17: -(dy^2+dx^2)/72 + ln(2*beta/z^2) - ln(rn_j)
    (row 17's rhs side, -ln(rn_j), is filled in on device after colsum)
    """
    ys = np.arange(P) // W
    xs = np.arange(P) % W
    # centering leaves all pairwise differences (and hence K, S) unchanged
    # but shrinks feature magnitudes so FP22-rounded products stay accurate
    yc, xc = ys - 47.5, xs - 47.5
    c = np.asarray(ref, np.float64).reshape(3, P) - 127.5

    g1 = np.exp(-((np.arange(71.0) - 35.0) ** 2) / 72.0)
    z = g1.sum()

    yf, xf = _m13(yc / SXY_BF), _m13(xc / SXY_BF)
    cf = _m13(c / SC_BF)
    syxC = _m13(-0.5 * (yf ** 2 + xf ** 2) + np.log(4.0) + np.log(BETA))
    syxQ = _m13(-0.5 * (yf ** 2 + xf ** 2))
    cd = _m13(-0.5 * cf ** 2)
    y6, x6 = _m13(yc / SXY_SP), _m13(xc / SXY_SP)
    sdC = _m13(-0.5 * (y6 ** 2 + x6 ** 2) + np.log(2.0 * BETA)
               - 2.0 * np.log(z))
    sdQ = _m13(-0.5 * (y6 ** 2 + x6 ** 2))
    one = np.ones(P)
    zero = np.zeros(P)

    pl = np.stack([yf, xf, *cf, syxC, *cd, one, one, one, one,
                   y6, x6, sdC, one, one], 0)
    qr = np.stack([yf, xf, *cf, one, one, one, one, syxQ, *cd,
                   y6, x6, one, sdQ, zero], 0)
    return (np.ascontiguousarray(pl, dtype=np.float32),
            np.ascontiguousarray(qr, dtype=np.float32))


# ----------------------------------------------------------------------------
# device program
# ----------------------------------------------------------------------------

def _build_bass(niters=5):
    key = ("nc", niters)
    if key in _CACHE:
        return _CACHE[key]

    import concourse.bass as bass
    import concourse.bacc as bacc
    import concourse.tile as tile
    import concourse.mybir as mybir
    from concourse.masks import make_identity

    f32 = mybir.dt.float32
    f32r = mybir.dt.float32r
    bf16 = mybir.dt.bfloat16
    fp8 = mybir.dt.float8e4
    AF = mybir.ActivationFunctionType
    ALU = mybir.AluOpType
    DR = mybir.MatmulPerfMode.DoubleRow

    LN4B = float(np.log(4.0 * BETA))
    LNBG = float(np.log(BETA * GAMMA))

    nc = bacc.Bacc("TRN2", num_devices=NCORES)

    pl = nc.dram_tensor("pl", [18, P], f32r, kind="ExternalInput")
    qr = nc.dram_tensor("qr", [18, PSH], f32r, kind="ExternalInput")
    unary_m = nc.dram_tensor("unary_m", [L, PSH], f32, kind="ExternalInput")
    qout = nc.dram_tensor("qout", [NJ, 128, L], f32, kind="ExternalOutput")

    rg = [list(range(NCORES))]

    with tile.TileContext(nc) as tc:
        with tc.tile_pool(name="dram", bufs=1, space="DRAM") as dram:
            rn_in_d = dram.tile([1, PSH], f32)      # my rn, AG input
            rn_out_d = dram.tile([NI, 128], f32)    # full rn, AG output
            rnp_d = dram.tile([1, PSH], f32)        # my rn_j/(beta*gamma)
            rnpi_d = dram.tile([1, PSH], f32)       # its reciprocal
            qsh_d = dram.tile([NJ, 128, L], fp8)    # q shard, AG input
            qfl_d = dram.tile([NI, 128, L], fp8)    # full q, AG output

            with tc.tile_pool(name="persist", bufs=1) as persist:
                asmQR = persist.tile([NROW1, PSH], f32r)
                nc.sync.dma_start(asmQR[:], qr[0:NROW1, :])
                asmQR2 = persist.tile([NROW2, PSH], f32r)
                nc.sync.dma_start(asmQR2[0:NROW2 - 1, :],
                                  qr[NROW1:NROW1 + NROW2 - 1, :])
                ident = persist.tile([NI, NI], f32)
                make_identity(nc, ident[:])
                U_sb = persist.tile([L, PSH], f32)
                U2 = persist.tile([L, PSH], f32)    # U / (rn_j/(beta*gamma))
                rnI = persist.tile([128, NI], f32)
                rnJpix = persist.tile([128, NJ], f32)
                # DoubleRow weights need pair-dim stride % 16 == 0
                ones2 = persist.tile([128, 2, 16], fp8)
                nc.vector.memset(ones2[:], 1.0)

                mpool = tc.tile_pool(name="mres", bufs=1)
                mpool_h = mpool.__enter__()
                Mt = mpool_h.tile([128, NP, 2, PSH], fp8, name="Mt")

                with (
                    tc.tile_pool(name="itq", bufs=1) as itq,
                    tc.tile_pool(name="ite", bufs=4) as ite,
                    tc.tile_pool(name="ittp", bufs=2, space="PSUM") as ittp,
                ):
                    # U = ln(clip(u))
                    ut = itq.tile([L, PSH], f32, tag="ut")
                    nc.sync.dma_start(ut[:], unary_m[:, :])
                    nc.vector.tensor_scalar(ut[:], ut[:], 1e-5, 1.0,
                                            op0=ALU.max, op1=ALU.min)
                    nc.scalar.activation(U_sb[:], ut[:], AF.Ln)

                    def softmax_stage(qh_cur, it, scaled=False):
                        last = it == niters
                        if not last:
                            qm = itq.tile([128, NJ, L], fp8, tag="qm")
                        else:
                            qo = itq.tile([128, NJ, L], f32, tag="qo")
                        zz = ite.tile([128, NJ], f32, tag="zz")
                        rz = ite.tile([128, NJ], f32, tag="rz")
                        for jc in range(NJ):
                            tp = ittp.tile([128, L], f32, tag="tp")
                            nc.tensor.transpose(
                                tp[:], qh_cur[:, jc * 128:(jc + 1) * 128],
                                ident[0:L, 0:L])
                            e = ite.tile([128, L], f32, tag="e")
                            # scaled: logits arrive pre-divided by
                            # rn_j/(beta*gamma); the ACT scale restores
                            # q_hat = ps*rnpost + U inside the exp
                            nc.scalar.activation(
                                e[:], tp[:], AF.Exp,
                                scale=(rnJpix[:, jc:jc + 1] if scaled
                                       else 1.0),
                                accum_out=zz[:, jc:jc + 1])
                            nc.vector.reciprocal(rz[:, jc:jc + 1],
                                                 zz[:, jc:jc + 1])
                            if not last:
                                nc.vector.tensor_scalar(
                                    qm[:, jc, :], e[:], rz[:, jc:jc + 1],
                                    GAMMA, op0=ALU.mult, op1=ALU.mult)
                            else:
                                nc.vector.tensor_scalar_mul(
                                    qo[:, jc, :], e[:], rz[:, jc:jc + 1])
                        if not last:
                            nc.gpsimd.dma_start(
                                qsh_d.rearrange("a b c -> b a c"), qm[:])
                            nc.gpsimd.collective_compute(
                                "AllGather", ALU.bypass, replica_groups=rg,
                                ins=[qsh_d.rearrange("a b c -> (a b c)")],
                                outs=[qfl_d.rearrange("a b c -> (a b c)")])
                            # L padded to 32 so the DoubleRow pair-dim
                            # stride (32) is a multiple of 16
                            qf = itq.tile([128, NI, 32], fp8, tag="qf",
                                          bufs=2)
                            nc.sync.dma_start(
                                qf[:, :, 0:L],
                                qfl_d.rearrange("a b c -> b a c"))
                            return qf
                        nc.gpsimd.dma_start(
                            qout[:, :, :].rearrange("a b c -> b a c"), qo[:])
                        return None

                    # iteration-0 softmax + AllGather overlap the build
                    qf_cur = softmax_stage(U_sb, 0)

                    # ---- pass 1: E = exp(T1) into the fp8 resident tile --
                    with (
                        tc.tile_pool(name="slab", bufs=2) as slabp,
                        tc.tile_pool(name="eps", bufs=2, space="PSUM") as eps,
                    ):
                        for sb in range(NSLAB):
                            sl = slabp.tile([NROW1, PSH], f32r, tag="sl")
                            nc.sync.dma_start(
                                sl[:], pl[0:NROW1, sb * PSH:(sb + 1) * PSH])
                            for k in range(NI // NSLAB):
                                ic = sb * (NI // NSLAB) + k
                                ps = eps.tile([128, PSH], f32, tag="eps")
                                lh = sl[:, k * 128:(k + 1) * 128]
                                for (o, n) in SLICES:
                                    nc.tensor.matmul(
                                        ps[:, o:o + n],
                                        lh,
                                        asmQR[0:NROW1, o:o + n],
                                        start=True, stop=True)
                                nc.scalar.activation(
                                    Mt[:, ic // 2, ic % 2, :], ps[:], AF.Exp)

                    # ---- colsum (DoubleRow over fp8) -> rn rows ----------
                    with (
                        tc.tile_pool(name="csp", bufs=1, space="PSUM") as csp,
                        tc.tile_pool(name="cst1", bufs=1) as cst1,
                    ):
                        cs = csp.tile([1, PSH], f32)
                        for k in range(NP):
                            for (o, n) in SLICES:
                                nc.tensor.matmul(cs[:, o:o + n],
                                                 ones2[:, :, 0:1],
                                                 Mt[:, k, :, o:o + n],
                                                 start=(k == 0),
                                                 stop=(k == NP - 1),
                                                 perf_mode=DR)
                        lncs = cst1.tile([1, PSH], f32)
                        nc.scalar.activation(lncs[:], cs[:], AF.Ln)
                        # rn AllGather feed first, so the collective starts
                        # as early as possible
                        rnrow = cst1.tile([1, PSH], f32)
                        nc.vector.tensor_scalar(rnrow[:], lncs[:],
                                                -0.5, 0.5 * LN4B,
                                                op0=ALU.mult, op1=ALU.add)
                        nc.scalar.activation(rnrow[:], rnrow[:], AF.Exp)
                        nc.gpsimd.dma_start(rn_in_d[:, :], rnrow[:])
                        nc.gpsimd.collective_compute(
                            "AllGather", ALU.bypass, replica_groups=rg,
                            ins=[rn_in_d.rearrange("a b -> (a b)")],
                            outs=[rn_out_d.rearrange("a b -> (a b)")])
                        # -ln(rn_j) = 0.5*ln(cs) - 0.5*ln(4*beta)
                        qrow17 = cst1.tile([1, PSH], f32)
                        nc.vector.tensor_scalar(qrow17[:], lncs[:],
                                                0.5, -0.5 * LN4B,
                                                op0=ALU.mult, op1=ALU.add)
                        nc.sync.dma_start(asmQR2[NROW2 - 1:NROW2, :],
                                          qrow17[:].bitcast(f32r))
                        rnpost = cst1.tile([1, PSH], f32)
                        nc.vector.tensor_scalar(rnpost[:], lncs[:],
                                                -0.5, 0.5 * LN4B - LNBG,
                                                op0=ALU.mult, op1=ALU.add)
                        nc.scalar.activation(rnpost[:], rnpost[:], AF.Exp)
                        nc.sync.dma_start(rnp_d[:, :], rnpost[:])
                        # 1/rnpost row, to pre-divide U for the ACT-scale
                        # softmax trick
                        rnpinv = cst1.tile([1, PSH], f32)
                        nc.vector.tensor_scalar(rnpinv[:], lncs[:],
                                                0.5, LNBG - 0.5 * LN4B,
                                                op0=ALU.mult, op1=ALU.add)
                        nc.scalar.activation(rnpinv[:], rnpinv[:], AF.Exp)
                        nc.sync.dma_start(rnpi_d[:, :], rnpinv[:])

                        # rn arrives [NI,128]; transpose on PE (contiguous
                        # DMA + transpose beats a 9216-element scattered DMA)
                        rnT = cst1.tile([NI, 128], f32)
                        nc.sync.dma_start(rnT[:], rn_out_d[:, :])
                        rnps = csp.tile([128, NI], f32, tag="rnps", bufs=1)
                        nc.tensor.transpose(rnps[:], rnT[:], ident[:])
                        nc.vector.tensor_copy(rnI[:], rnps[:])
                        nc.sync.dma_start(
                            rnJpix[:],
                            rnp_d.rearrange("a (j t) -> (a t) j", t=128))
                        rnpinv21 = cst1.tile([L, PSH], f32)
                        nc.sync.dma_start(
                            rnpinv21[:],
                            rnpi_d[0:1, :].to_broadcast((L, PSH)))
                        nc.vector.tensor_tensor(U2[:], U_sb[:], rnpinv21[:],
                                                op=ALU.mult)

                    # ---- pass 2: Mt = (E * rn_i) + exp(T2') --------------
                    # T2' matmuls/exp depend only on the local colsum (via
                    # asmQR row 17); only the STT waits for the rn AllGather
                    with (
                        tc.tile_pool(name="slab2", bufs=2) as slab2,
                        tc.tile_pool(name="ps2p", bufs=2,
                                     space="PSUM") as ps2p,
                        tc.tile_pool(name="e2p", bufs=3) as e2p,
                    ):
                        for sb in range(NSLAB):
                            sl2 = slab2.tile([NROW2, PSH], f32r, tag="sl2")
                            nc.sync.dma_start(
                                sl2[:],
                                pl[NROW1:NROW1 + NROW2,
                                   sb * PSH:(sb + 1) * PSH])
                            for k in range(NI // NSLAB):
                                ic = sb * (NI // NSLAB) + k
                                ps2 = ps2p.tile([128, PSH], f32, tag="ps2")
                                lh2 = sl2[:, k * 128:(k + 1) * 128]
                                for (o, n) in SLICES:
                                    nc.tensor.matmul(
                                        ps2[:, o:o + n],
                                        lh2,
                                        asmQR2[:, o:o + n],
                                        start=True, stop=True)
                                e2 = e2p.tile([128, PSH], bf16, tag="e2")
                                nc.scalar.activation(e2[:], ps2[:], AF.Exp)
                                mt_c = Mt[:, ic // 2, ic % 2, :]
                                # Pool is ~3.3x slower per chunk (2 ops) so
                                # it gets a quarter of the chunks
                                if ic % 4 < 3:
                                    nc.vector.scalar_tensor_tensor(
                                        mt_c, mt_c, rnI[:, ic:ic + 1], e2[:],
                                        op0=ALU.mult, op1=ALU.add)
                                else:
                                    nc.gpsimd.tensor_scalar_mul(
                                        mt_c, mt_c, rnI[:, ic:ic + 1])
                                    nc.gpsimd.tensor_add(mt_c, mt_c, e2[:])

                    # ---- iterations (DoubleRow matvec) -------------------
                    with tc.tile_pool(name="itps", bufs=1,
                                      space="PSUM") as itps:
                        for it in range(1, niters + 1):
                            ps = itps.tile([L, PSH], f32, tag="qbps")
                            for k in range(NP):
                                lhq = qf_cur[:, 2 * k:2 * k + 2, 0:L]
                                for (o, n) in SLICES:
                                    nc.tensor.matmul(
                                        ps[:, o:o + n], lhq,
                                        Mt[:, k, :, o:o + n],
                                        start=(k == 0),
                                        stop=(k == NP - 1),
                                        perf_mode=DR)
                            qh = itq.tile([L, PSH], f32, tag="qh", bufs=2)
                            nc.vector.tensor_add(qh[:], ps[:], U2[:])
                            qf_cur = softmax_stage(qh, it, scaled=True)
                mpool.__exit__(None, None, None)

    nc.finalize()
    _CACHE[key] = nc
    return nc


# ----------------------------------------------------------------------------
# host entry point
# ----------------------------------------------------------------------------

def _in_maps(unary, ref):
    plf, qrf = _host_rows(ref)
    u2 = np.ascontiguousarray(np.asarray(unary, np.float32).reshape(L, P))
    maps = []
    for c in range(NCORES):
        sl = slice(c * PSH, (c + 1) * PSH)
        maps.append({
            "pl": plf,
            "qr": np.ascontiguousarray(qrf[:, sl]),
            "unary_m": np.ascontiguousarray(u2[:, sl]),
        })
    return maps


def kernel(unary: np.ndarray, ref: np.ndarray) -> np.ndarray:
    from concourse import bass_utils

    nc = _build_bass()
    in_maps = _in_maps(unary, ref)

    global LAST_RESULT
    res = bass_utils.run_bass_kernel_spmd(nc, in_maps,
                                          core_ids=list(range(NCORES)),
                                          trace=TRACE)
    LAST_RESULT = res
    shards = [res.results[c]["qout"].reshape(PSH, L) for c in range(NCORES)]
    qfull = np.concatenate(shards, 0)          # [P, L]
    out = qfull.T.reshape(1, L, H, W).astype(np.float32)
    return out


if __name__ == "__main__":
    u = np.random.rand(1, L, H, W).astype(np.float32)
    r = (np.random.rand(1, 3, H, W) * 255).astype(np.float32)
    o = kernel(u, r)
    print(o.shape, o.dtype, o.sum())
